# revision 14
# baseline (speedup 1.0000x reference)
"""Committee-vote histogram kernel for TRN2 (8 NeuronCores, data-parallel).

votes[b, c] = sum_m 1[argmax_c' (x[b] @ W[m, :, c'] + b[m, c']) == c]

Strategy per core (batch shard of 8192 rows):
  - x tile [128b, 256d] loaded contiguously; PE fp32 transpose-mode -> xT in
    PSUM; ScalarE copies xT to SBUF; PE fp32 matmuls accumulate
    logits [128b, 80(m,c)] in PSUM; DVE does bias-add, per-(m) max over the 10
    classes, broadcast-compare (is_ge), and per-(c) sum over the 8 members.
  - W/b are tiny and replicated to every core.
"""

import os
import sys

import numpy as np

if os.path.isdir("/opt/trn_rl_repo") and "/opt/trn_rl_repo" not in sys.path:
    sys.path.insert(0, "/opt/trn_rl_repo")

import concourse.bass as bass
import concourse.tile as tile
from concourse import bacc, mybir
from concourse.bass import ts
from concourse.bass_utils import run_bass_kernel_spmd
from concourse.masks import make_identity

F32 = mybir.dt.float32

B_FULL = 65536
D = 256
C = 10
M = 8
N_CORES = 8
B_SHARD = B_FULL // N_CORES  # 8192
P = 128

MC = M * C  # 80 logit columns per sample

# float32r (single-pass matmul) is lossy — the BIR verifier requires
# producers rounded to fp32r, i.e. truncated mantissas: unacceptable for
# argmax fidelity. Stay on full fp32 (two half-speed passes).
MM_DTYPE = mybir.dt.float32
BF16 = mybir.dt.bfloat16


def build_nc(b_shard: int = B_SHARD) -> bass.Bass:
    n_tiles = b_shard // P  # 128-row tiles
    # group = unit of input DMA (16 tiles = 2 MB); batch = unit of DVE vote ops
    tiles_per_group = min(16, n_tiles)
    n_groups = n_tiles // tiles_per_group
    assert n_tiles % tiles_per_group == 0
    batches_per_group = tiles_per_group // 4  # 4 tiles per vote batch

    nc = bacc.Bacc("TRN2", target_bir_lowering=False)
    x = nc.dram_tensor("x", [b_shard, D], F32, kind="ExternalInput")
    w = nc.dram_tensor("w", [D, MC], F32, kind="ExternalInput")
    bv = nc.dram_tensor("b", [1, MC], F32, kind="ExternalInput")
    y = nc.dram_tensor("y", [b_shard, C], F32, kind="ExternalOutput")

    with tile.TileContext(nc) as tc:
        with (
            tc.tile_pool(name="consts", bufs=1) as consts,
            tc.tile_pool(name="xg", bufs=2) as xg_pool,
            tc.tile_pool(name="xt", bufs=3) as xt_pool,
            tc.tile_pool(name="pxt", bufs=3, space="PSUM") as pxt_pool,
            tc.tile_pool(name="lg", bufs=3, space="PSUM") as lg_pool,
            tc.tile_pool(name="scr", bufs=1, space="PSUM") as scr_pool,
            tc.tile_pool(name="mx", bufs=2) as mx_pool,
            tc.tile_pool(name="eq", bufs=2) as eq_pool,
            tc.tile_pool(name="stg", bufs=2) as stg_pool,
        ):
            ident = consts.tile([P, P], F32)
            make_identity(nc, ident)

            # W as [128 d', k, 80] where d = 128k + d'
            w_sb = consts.tile([P, 2, MC], F32)
            nc.sync.dma_start(w_sb, w.rearrange("(k p) c -> p k c", p=P))

            # bias, exactly decomposed into a bf16 pair (b = hi + lo + O(2^-17)),
            # replicated x4 along the free dim; added to logits by seeding the
            # PSUM accumulation group with two K=1 bf16 matmuls against ones.
            brep = consts.tile([P, MC], F32)
            nc.sync.dma_start(brep, bv[:].broadcast_to([P, MC]))
            bhi = consts.tile([1, 4 * MC], BF16)
            blo_f = consts.tile([1, MC], F32)
            blo = consts.tile([1, 4 * MC], BF16)
            ones_row = consts.tile([1, P], BF16)
            nc.vector.memset(ones_row, 1.0)
            for r in range(4):
                nc.vector.tensor_copy(bhi[:, ts(r, MC)], brep[0:1, :])
            nc.vector.tensor_tensor(
                out=blo_f, in0=brep[0:1, :], in1=bhi[0:1, 0:MC],
                op=mybir.AluOpType.subtract,
            )
            for r in range(4):
                nc.vector.tensor_copy(blo[:, ts(r, MC)], blo_f)

            # PE fence instructions: absorb the one-time identity (gpsimd) and
            # w_sb (DMA) deps into cheap throwaway PE ops.
            scr = scr_pool.tile([P, P], F32)
            nc.tensor.transpose(scr, ident, ident)
            nc.tensor.matmul(
                scr[:MC, 0:1], lhsT=w_sb[:, 0, :], rhs=w_sb[:, 0, 0:1],
                start=True, stop=True,
            )

            for g in range(n_groups):
                xg = xg_pool.tile([P, tiles_per_group, D], F32)
                nc.sync.dma_start(
                    xg,
                    x[g * tiles_per_group * P : (g + 1) * tiles_per_group * P, :]
                    .rearrange("(t p) d -> p t d", p=P),
                )
                stg = stg_pool.tile([P, tiles_per_group * C], F32)

                for bi in range(batches_per_group):
                    lg = lg_pool.tile([P, 4 * MC], F32)  # logits, 4 tiles
                    # seed the accumulation group with the bias (every row of
                    # ones^T @ b_rep is b_rep); also absorbs the lg-slot
                    # release wait so later matmuls carry only their ACT wait
                    nc.tensor.matmul(
                        lg, lhsT=ones_row, rhs=bhi, start=True, stop=False
                    )
                    nc.tensor.matmul(
                        lg, lhsT=ones_row, rhs=blo, start=False, stop=False
                    )
                    for pj in range(2):  # 2 tile-pairs per batch
                        t0 = bi * 4 + pj * 2
                        pxt = pxt_pool.tile([P, 512], F32)
                        for tt in range(2):
                            for k in range(2):
                                nc.tensor.transpose(
                                    pxt[:, tt * 256 + k * P : tt * 256 + (k + 1) * P],
                                    xg[:, t0 + tt, k * P : (k + 1) * P],
                                    ident,
                                )
                        xts = xt_pool.tile([P, 512], F32)
                        nc.scalar.copy(xts, pxt)
                        for tt in range(2):
                            j = pj * 2 + tt
                            for k in range(2):
                                nc.tensor.matmul(
                                    lg[:, j * MC : (j + 1) * MC],
                                    lhsT=xts[
                                        :, tt * 256 + k * P : tt * 256 + (k + 1) * P
                                    ].bitcast(MM_DTYPE),
                                    rhs=w_sb[:, k, :].bitcast(MM_DTYPE),
                                    start=False,
                                    stop=(j == 3 and k == 1),
                                )

                    # votes for this 4-tile batch (logits read from PSUM)
                    mx = mx_pool.tile([P, 4 * M], F32)
                    nc.vector.reduce_max(
                        mx,
                        lg[:].rearrange("p (a c) -> p a c", c=C),
                        axis=mybir.AxisListType.X,
                    )
                    eq = eq_pool.tile([P, 4 * MC], F32)
                    nc.vector.tensor_tensor(
                        out=eq[:].rearrange("p (a c) -> p a c", c=C),
                        in0=lg[:].rearrange("p (a c) -> p a c", c=C),
                        in1=mx[:, :, None].broadcast_to([P, 4 * M, C]),
                        op=mybir.AluOpType.is_ge,
                    )
                    nc.vector.reduce_sum(
                        stg[:, ts(bi, 4 * C)],
                        eq[:].rearrange("p (t m c) -> p t c m", t=4, m=M, c=C),
                        axis=mybir.AxisListType.X,
                    )

                nc.sync.dma_start(
                    y[g * tiles_per_group * P : (g + 1) * tiles_per_group * P, :]
                    .rearrange("(t p) c -> p t c", p=P),
                    stg[:].rearrange("p (t c) -> p t c", c=C),
                )
    nc.compile()
    return nc


_NC_CACHE: dict[int, bass.Bass] = {}


def _get_nc(b_shard: int) -> bass.Bass:
    if b_shard not in _NC_CACHE:
        _NC_CACHE[b_shard] = build_nc(b_shard)
    return _NC_CACHE[b_shard]


def kernel(x: np.ndarray, W: np.ndarray, b: np.ndarray, **_) -> np.ndarray:
    x = np.ascontiguousarray(np.asarray(x, dtype=np.float32))
    assert x.shape == (B_FULL, D), x.shape
    # m-major columns: col index = 10*m + c (matches b.reshape order)
    wf = np.ascontiguousarray(
        np.asarray(W, dtype=np.float32).transpose(1, 0, 2).reshape(D, MC)
    )
    bf = np.ascontiguousarray(np.asarray(b, dtype=np.float32).reshape(1, MC))

    nc = _get_nc(B_SHARD)
    in_maps = [
        {"x": x[i * B_SHARD : (i + 1) * B_SHARD], "w": wf, "b": bf}
        for i in range(N_CORES)
    ]
    res = run_bass_kernel_spmd(nc, in_maps, core_ids=list(range(N_CORES)))
    return np.concatenate([res.results[i]["y"] for i in range(N_CORES)], axis=0)
